# revision 1
# baseline (speedup 1.0000x reference)
"""Cluster-GCN layer on 8 Trainium2 NeuronCores (Bass/Tile).

Math (see reference): with A_norm the intra-cluster normalized adjacency and
deg = intra-in-degree + 1,

    out = A_norm @ (X W) + diag(1/deg) (X W) + b        (masked rows keep X)
        = (X + (diag(1/deg) - I) X_recv + A_norm X) @ W + b

Sharding: clusters are greedily assigned to 8 cores, so intra-cluster edges
are core-local.  Per core, nodes get local column ids with RECEIVING nodes
(intra-in-degree > 0, ~17% of nodes) first, rank-ordered by unique in-degree
descending.  Shipped per core:

  x_ft    [128, T*128]     feature-major X^T: bulk-loaded at line rate,
                           streamed as matmul moving operand (W stationary).
  gtab    [128*GKT, D]     edge rounds' source rows (round r>=1 slot k holds
                           the r-th unique in-edge source of receiving node
                           k), pre-gathered on the host (index plumbing
                           only), one line-rate DMA - no indirect DMA
                           anywhere.
  smalls  [128, 129+RT+..] W | b | per-slot degree counts & multiplicities,
                           one DMA.

The compact correction  Z = (diag(1/deg)-I) X_recv + (A_norm X)_recv  is
built by per-partition scaling of the round buffers (the self term's rows
are recovered from x_ft columns with a PE transpose; the device computes
1/deg and 1/sqrt(deg) itself), accumulated across rounds directly in PSUM by
PE transposes, and applied by extending the PSUM matmul group of the first
column chunks with  += W^T @ Z^T  (receiving nodes occupy the first columns
by construction, so no scatter is needed anywhere).

Device does all float math; host does integer/index preprocessing and data
layout only.
"""

import numpy as np

import concourse.bacc as bacc
import concourse.mybir as mybir
import concourse.tile as tile
from concourse.bass_utils import run_bass_kernel_spmd
from concourse.masks import make_identity

N_CORES = 8
P = 128           # partitions
D = 128           # feature dim
N_CLUSTERS = 64
LOAD_TILES = 4    # node tiles per bulk DMA chunk (2KB/partition, 1 matmul)
MM_COLS = 512     # moving-operand columns per matmul (one PSUM bank)
MM_SPLIT = 3      # main matmuls emitted before the Z/correction pipeline
WARMUP_MM = 8     # scratch matmuls to ramp the PE clock during DMA-in
GTAB_POS = 4      # x-chunk after which the gather table is queued

F32 = mybir.dt.float32


# --------------------------------------------------------------------------
# Bass program (SPMD across cores; one program, per-core data)
# --------------------------------------------------------------------------

def build_program(T, RT, KTS, has_bias, mask_cols):
    """T: node tiles; RT: receiver tiles; KTS: per-round tile counts
    (round 0 = self term, kt=RT; rounds 1.. = edge rounds); mask_cols:
    trailing columns that must keep raw X (0 = none)."""
    R = len(KTS)
    NC = T * P
    GKT = sum(KTS[1:])      # gather-table tiles (edge rounds only; the self
                            # round's rows are recovered from x_ft on-chip)
    # smalls layout: W (128 cols) | b (1 col, if bias) | degd (RT) |
    #                per round r>=1: wm_r (kt) | degs_r (kt)
    s_cols = D + (1 if has_bias else 0) + RT + 2 * GKT
    nc = bacc.Bacc("TRN2", target_bir_lowering=False, debug=False)

    x_ft = nc.declare_dram_parameter("x_ft", [P, NC], F32, isOutput=False)
    smalls = nc.declare_dram_parameter("smalls", [P, s_cols], F32, isOutput=False)
    if GKT:
        gtab = nc.declare_dram_parameter("gtab", [P * GKT, D], F32, isOutput=False)
    out_ft = nc.declare_dram_parameter("out_ft", [P, NC], F32, isOutput=True)

    n_ch = (T + LOAD_TILES - 1) // LOAD_TILES          # load/store chunks
    ch_cols = [min(LOAD_TILES, T - c * LOAD_TILES) * P for c in range(n_ch)]
    zc = RT * P                                         # correction columns

    with tile.TileContext(nc) as tc:
        with (
            tc.tile_pool(name="const", bufs=1) as cpool,
            tc.tile_pool(name="xbuf", bufs=1) as xpool,
            tc.tile_pool(name="stage", bufs=1) as spool,
            tc.tile_pool(name="gbuf", bufs=1) as gpool,
            tc.tile_pool(name="tmp", bufs=4) as mpool,
            tc.tile_pool(name="zt", bufs=1) as zpool,
            tc.tile_pool(name="mmp", bufs=4, space="PSUM") as mpsum,
            tc.tile_pool(name="trp", bufs=2, space="PSUM") as tpsum,
        ):
            # ---- packed small inputs via SWDGE ----
            sm_sb = cpool.tile([P, s_cols], F32, tag="smalls")
            nc.gpsimd.dma_start(out=sm_sb[:], in_=smalls[:])
            ident = cpool.tile([P, P], F32, tag="ident")
            make_identity(nc, ident[:])

            # ---- PE warmup: cheap matmuls on scratch during the initial
            #      DMA window, so real matmuls run at full clock ----
            wu = cpool.tile([P, P], F32, tag="wu")
            nc.vector.memset(wu[:], 1.0)
            for _ in range(WARMUP_MM):
                wu_ps = tpsum.tile([P, P], F32, tag="xtp")
                nc.tensor.matmul(
                    out=wu_ps[:], lhsT=wu[:], rhs=wu[:], start=True, stop=True
                )

            w_sb = sm_sb[:, 0:D]
            off = D
            if has_bias:
                b_sb = sm_sb[:, off:off + 1]
                off += 1
            degd_sb = sm_sb[:, off:off + RT]
            off += RT
            wm_sb, degs_sb = [None], [None]
            for r in range(1, R):
                kt = KTS[r]
                wm_sb.append(sm_sb[:, off:off + kt]); off += kt
                degs_sb.append(sm_sb[:, off:off + kt]); off += kt

            g_off = [sum(KTS[1:r]) for r in range(R)]   # tile offset per round

            def g_tile(r, k):
                o = (g_off[r] + k) * P
                return g_all[:, o:o + P]

            # ---- bulk X^T load (chunked, line-rate, HWDGE) ----
            # Transfer order: the first-wave matmul chunks first; the
            # correction-region chunks (consumed late by the Z path) and the
            # gather table last, so the PE never starves mid-stream.
            n_zmm = (zc + MM_COLS - 1) // MM_COLS      # mm chunks with corr
            zch = min((zc + LOAD_TILES * P - 1) // (LOAD_TILES * P), n_ch)
            x_ch = [None] * n_ch
            g_all = None
            for c in range(n_ch):
                xt = xpool.tile([P, LOAD_TILES * P], F32, tag=f"x{c}")
                c0 = c * LOAD_TILES * P
                # correction chunks (needed mid-kernel by the Z path) ride
                # the otherwise-idle SWDGE queue; the sync stream leads with
                # the first-wave matmul chunks
                eng = nc.gpsimd if c < zch else nc.sync
                eng.dma_start(
                    out=xt[:, :ch_cols[c]], in_=x_ft[:, c0:c0 + ch_cols[c]]
                )
                x_ch[c] = xt
                if GKT and c == min(GTAB_POS, n_ch - 1):
                    g_all = gpool.tile([P, GKT * P], F32, tag="gall")
                    nc.sync.dma_start(
                        out=g_all[:],
                        in_=gtab.rearrange("(p r) f -> p (r f)", p=P),
                    )

            # ---- per-slot weights ----
            w_rounds = []
            if RT:
                d1 = mpool.tile([P, RT], F32, tag="wprep")
                nc.vector.tensor_scalar_add(d1[:], degd_sb, 1.0)
                dinv = cpool.tile([P, RT], F32, tag="dinv")
                nc.vector.reciprocal(dinv[:], d1[:])
                wex = cpool.tile([P, RT], F32, tag="wex")
                nc.vector.tensor_scalar_add(wex[:], dinv[:], -1.0)
                w_rounds.append(wex)
                wd = cpool.tile([P, RT], F32, tag="wd")
                nc.scalar.sqrt(wd[:], dinv[:])
                for r in range(1, R):
                    kt = KTS[r]
                    s1 = mpool.tile([P, kt], F32, tag="wprep")
                    nc.vector.tensor_scalar_add(s1[:], degs_sb[r], 1.0)
                    rec = mpool.tile([P, kt], F32, tag="wprep")
                    nc.vector.reciprocal(rec[:], s1[:])
                    ws = mpool.tile([P, kt], F32, tag="wprep")
                    nc.scalar.sqrt(ws[:], rec[:])
                    wr = cpool.tile([P, kt], F32, tag=f"wr{r}")
                    nc.vector.tensor_mul(wr[:], wm_sb[r], ws[:])
                    nc.vector.tensor_mul(wr[:], wr[:], wd[:, :kt])
                    w_rounds.append(wr)

            staging = []
            for c in range(n_ch):
                st = spool.tile([P, LOAD_TILES * P], F32, tag=f"s{c}")
                staging.append(st)

            def evict(c_mm, ps, w_):
                """PSUM -> staging for mm chunk c_mm, alternating DVE/ACT."""
                ch = (c_mm * MM_COLS) // (LOAD_TILES * P)
                o = c_mm * MM_COLS - ch * LOAD_TILES * P
                dst = staging[ch][:, o:o + w_]
                if has_bias:
                    if c_mm % 2 == 0:
                        nc.vector.tensor_scalar_add(dst, ps[:, :w_], b_sb)
                    else:
                        nc.scalar.add(dst, ps[:, :w_], b_sb)
                else:
                    if c_mm % 2 == 0:
                        nc.vector.tensor_copy(dst, ps[:, :w_])
                    else:
                        nc.scalar.copy(dst, ps[:, :w_])

            def mm_rhs(c_mm, w_):
                c0 = c_mm * MM_COLS
                ch = c0 // (LOAD_TILES * P)
                o = c0 - ch * LOAD_TILES * P
                return x_ch[ch][:, o:o + w_]

            n_mm = (NC + MM_COLS - 1) // MM_COLS

            def main_mm(c):
                w_ = min(MM_COLS, NC - c * MM_COLS)
                ps = mpsum.tile([P, MM_COLS], F32, tag="mm")
                nc.tensor.matmul(
                    out=ps[:, :w_], lhsT=w_sb, rhs=mm_rhs(c, w_),
                    start=True, stop=True,
                )
                evict(c, ps, w_)

            # ---- first wave of main matmuls (while gather table lands) ----
            split = min(n_zmm + MM_SPLIT, n_mm)
            for c in range(n_zmm, split):
                main_mm(c)

            # ---- Z^T: scale rounds per-partition, transpose-accumulate ----
            # Self term: receiving nodes' rows are x_ft columns [0, zc) -
            # recover them node-major with a PE transpose instead of a
            # shipped gather table.
            zt_sb = None
            if RT:
                zt_sb = zpool.tile([P, zc], F32, tag="zt")
                for k in range(RT):
                    terms = [r for r in range(R) if k < KTS[r]]
                    zp = tpsum.tile([P, P], F32, tag="ztp")
                    for i, r in enumerate(terms):
                        sg = mpool.tile([P, P], F32, tag="sg")
                        if r == 0:
                            xp = tpsum.tile([P, P], F32, tag="xtp")
                            ch = k // LOAD_TILES
                            o = (k - ch * LOAD_TILES) * P
                            nc.tensor.transpose(
                                out=xp[:], in_=x_ch[ch][:, o:o + P],
                                identity=ident[:],
                            )
                            nc.vector.tensor_scalar_mul(
                                sg[:], xp[:], w_rounds[0][:, k:k + 1]
                            )
                        else:
                            nc.vector.tensor_scalar_mul(
                                sg[:], g_tile(r, k), w_rounds[r][:, k:k + 1]
                            )
                        nc.tensor.matmul(
                            out=zp[:], lhsT=sg[:], rhs=ident[:],
                            is_transpose=True,
                            start=(i == 0), stop=(i == len(terms) - 1),
                        )
                    nc.scalar.copy(zt_sb[:, k * P:(k + 1) * P], zp[:])

            # ---- correction chunks: PSUM group = W^T X^T + W^T Z^T (+b) ----
            for c in range(n_zmm):
                w_ = min(MM_COLS, NC - c * MM_COLS)
                zw = min(MM_COLS, zc - c * MM_COLS)
                ps = mpsum.tile([P, MM_COLS], F32, tag="mm")
                nc.tensor.matmul(
                    out=ps[:, :w_], lhsT=w_sb, rhs=mm_rhs(c, w_),
                    start=True, stop=False,
                )
                nc.tensor.matmul(
                    out=ps[:, :zw], lhsT=w_sb,
                    rhs=zt_sb[:, c * MM_COLS:c * MM_COLS + zw],
                    start=False, stop=True,
                )
                evict(c, ps, w_)

            # ---- remaining main matmuls ----
            for c in range(split, n_mm):
                main_mm(c)

            # ---- masked trailing columns keep raw X ----
            if mask_cols:
                m0 = NC - mask_cols
                ch = m0 // (LOAD_TILES * P)
                for c in range(ch, n_ch):
                    o0 = max(m0 - c * LOAD_TILES * P, 0)
                    nc.vector.tensor_copy(
                        staging[c][:, o0:ch_cols[c]], x_ch[c][:, o0:ch_cols[c]]
                    )

            # ---- bulk output store, in evict-completion order (SP FIFO
            #      is head-of-line blocking) ----
            fw = list(range(zch, min(zch + MM_SPLIT, n_ch)))
            rest = [c for c in range(n_ch) if c not in fw and c >= zch]
            store_order = fw + rest + list(range(zch))
            for c in store_order:
                c0 = c * LOAD_TILES * P
                nc.sync.dma_start(
                    out=out_ft[:, c0:c0 + ch_cols[c]],
                    in_=staging[c][:, :ch_cols[c]],
                )

    nc.finalize()
    return nc


# --------------------------------------------------------------------------
# Host-side sharding / index preprocessing (integer ops + layout only)
# --------------------------------------------------------------------------

def _prepare(X, W, b, cluster_assignment, edge_index):
    N = X.shape[0]
    has_bias = bool(np.any(b))
    ca = np.asarray(cluster_assignment).astype(np.int64)
    ei = np.asarray(edge_index).astype(np.int64)
    n_cl = max(N_CLUSTERS, int(ca.max()) + 1 if ca.size else 1)
    src, dst = ei[0], ei[1]
    intra = ca[src] == ca[dst]
    isrc, idst = src[intra], dst[intra]

    degcnt = np.bincount(idst, minlength=N).astype(np.int64)
    cluster_edges = np.bincount(ca[isrc], minlength=n_cl)
    cluster_has = cluster_edges > 0
    node_masked = ~cluster_has[ca]          # rows that keep raw X
    any_mask = bool(node_masked.any())

    # dedup multi-edges -> (usrc, udst, mult)
    if len(isrc):
        pair = isrc * N + idst
        upair, mult = np.unique(pair, return_counts=True)
        usrc, udst = upair // N, upair % N
    else:
        usrc = udst = mult = np.zeros(0, dtype=np.int64)
    udeg = np.bincount(udst, minlength=N).astype(np.int64)

    # greedy cluster -> core assignment (balance node counts)
    csize = np.bincount(ca, minlength=n_cl)
    order = np.argsort(-csize, kind="stable")
    loads = np.zeros(N_CORES, dtype=np.int64)
    cl_core = np.zeros(n_cl, dtype=np.int64)
    for c in order:
        k = int(loads.argmin())
        cl_core[c] = k
        loads[k] += csize[c]
    node_core = cl_core[ca]

    T = int(np.ceil(loads.max() / P))

    # per-core local node order: [recv by udeg desc][nonrecv unmasked]
    # ... [gap pads][masked]  (masked tail only exists when any_mask)
    cores = []
    max_nrecv = 0
    max_rounds = 0
    max_masked = 0
    for k in range(N_CORES):
        nodes_k = np.where(node_core == k)[0]
        deg_k = udeg[nodes_k]
        recv = nodes_k[deg_k > 0]
        recv = recv[np.argsort(-udeg[recv], kind="stable")]
        nonrecv = nodes_k[deg_k == 0]
        if any_mask:
            nr_masked = nonrecv[node_masked[nonrecv]]
            nonrecv = nonrecv[~node_masked[nonrecv]]
        else:
            nr_masked = np.zeros(0, dtype=np.int64)
        max_nrecv = max(max_nrecv, len(recv))
        max_masked = max(max_masked, len(nr_masked))
        if len(recv):
            max_rounds = max(max_rounds, int(udeg[recv].max()))
        cores.append(dict(recv=recv, nonrecv=nonrecv, masked=nr_masked))

    if any_mask:
        # every core needs >= max_masked trailing (pad+masked) slots
        for k in range(N_CORES):
            ck = cores[k]
            used = len(ck["recv"]) + len(ck["nonrecv"])
            while used + max_masked > T * P:
                T += 1

    RT = int(np.ceil(max_nrecv / P)) if max_nrecv else 0
    R = (1 + max_rounds) if RT else 0      # round 0 = self term

    # per-round tile counts (unified across cores); round 0 covers all recv
    KTS = [RT] if RT else []
    for r in range(1, R):
        m_r = 0
        for k in range(N_CORES):
            m_r = max(m_r, int((udeg[cores[k]["recv"]] > r - 1).sum()))
        KTS.append(int(np.ceil(m_r / P)))
    GKT = sum(KTS[1:])

    Xf = np.ascontiguousarray(np.asarray(X, dtype=np.float32))
    Wf = np.ascontiguousarray(np.asarray(W, dtype=np.float32))
    bf = np.asarray(b, dtype=np.float32).reshape(-1)
    in_maps = []
    for k in range(N_CORES):
        ck = cores[k]
        recv, nonrecv, masked = ck["recv"], ck["nonrecv"], ck["masked"]
        n_recv = len(recv)
        NCk = T * P
        # local (column) ids
        order_head = np.concatenate([recv, nonrecv])
        lid = np.full(N, -1, dtype=np.int64)
        lid[order_head] = np.arange(len(order_head))
        if len(masked):
            lid[masked] = NCk - len(masked) + np.arange(len(masked))
        ck["lid"] = lid
        ck["local_nodes"] = np.concatenate([order_head, masked])

        x_loc = np.zeros((NCk, D), dtype=np.float32)
        x_loc[lid[ck["local_nodes"]]] = Xf[ck["local_nodes"]]
        m = dict(x_ft=np.ascontiguousarray(x_loc.T))

        # smalls: W | b | degd | per-round wm, degs
        sm = [Wf, bf[:, None]] if has_bias else [Wf]
        gt = np.zeros((P, GKT, D), dtype=np.float32)   # [p, tile, feat]
        if RT:
            dd = np.zeros((P, RT), dtype=np.float32)
            ranks = np.arange(n_recv)
            pp0, tt0 = ranks % P, ranks // P
            dd[pp0, tt0] = degcnt[recv].astype(np.float32)
            sm.append(dd)

        # unique intra edges whose dst lives on this core
        sel = node_core[udst] == k
        es, ed, em = usrc[sel], udst[sel], mult[sel]
        rank_of = np.full(N, -1, dtype=np.int64)
        rank_of[recv] = np.arange(n_recv)
        rnk = rank_of[ed]
        o = np.argsort(rnk, kind="stable")
        es, em, rnk = es[o], em[o], rnk[o]
        if len(rnk):
            starts = np.r_[0, np.flatnonzero(np.diff(rnk)) + 1]
            grp = np.repeat(np.arange(len(starts)), np.diff(np.r_[starts, len(rnk)]))
            seq = np.arange(len(rnk)) - starts[grp]
        else:
            seq = np.zeros(0, dtype=np.int64)

        g_off = [sum(KTS[1:r]) for r in range(R)]
        for r in range(1, R):
            kt = KTS[r]
            wm = np.zeros((P, kt), dtype=np.float32)
            dg = np.zeros((P, kt), dtype=np.float32)
            e_r = seq == (r - 1)
            rr = rnk[e_r]
            pp, tt = rr % P, rr // P
            gt[pp, g_off[r] + tt] = Xf[es[e_r]]
            wm[pp, tt] = em[e_r].astype(np.float32)
            dg[pp, tt] = degcnt[es[e_r]].astype(np.float32)
            sm.append(wm)
            sm.append(dg)

        m["smalls"] = np.ascontiguousarray(np.concatenate(sm, axis=1))
        if GKT:
            m["gtab"] = np.ascontiguousarray(gt.reshape(P * GKT, D))
        in_maps.append(m)

    meta = dict(T=T, RT=RT, KTS=KTS, mask_cols=max_masked if any_mask else 0,
                cores=cores, N=N, has_bias=has_bias)
    return in_maps, meta


def _finish(results, meta):
    N = meta["N"]
    out = np.zeros((N, D), dtype=np.float32)
    for k in range(N_CORES):
        ck = meta["cores"][k]
        nodes = ck["local_nodes"]
        rows = ck["lid"][nodes]
        out[nodes] = results[k]["out_ft"].T[rows]
    return out


def _run(inputs, trace=False, trace_kwargs=None):
    X = np.asarray(inputs["X"], dtype=np.float32)
    W = np.asarray(inputs["W"], dtype=np.float32)
    b = np.asarray(inputs["b"], dtype=np.float32)
    in_maps, meta = _prepare(
        X, W, b, inputs["cluster_assignment"], inputs["edge_index"]
    )
    nc = build_program(meta["T"], meta["RT"], meta["KTS"], meta["has_bias"],
                       meta["mask_cols"])
    res = run_bass_kernel_spmd(
        nc, in_maps, list(range(N_CORES)), trace=trace,
        **(dict(trace_kwargs=trace_kwargs) if trace_kwargs else {}),
    )
    out = _finish(res.results, meta)
    return out, res


def kernel(**inputs) -> np.ndarray:
    out, _ = _run(inputs)
    return out



# revision 8
# speedup vs baseline: 1.2917x; 1.2917x over previous
"""Cluster-GCN layer on 8 Trainium2 NeuronCores (Bass/Tile), bf16 edition.

Math (see reference): with A_norm the intra-cluster normalized adjacency and
deg = intra-in-degree + 1,

    out = A_norm @ (X W) + diag(1/deg) (X W) + b        (masked rows keep X)
        = (X' ) @ W + b   where  X'        = X                  (non-recv cols)
                                 X'_recv   = diag(1/deg) X_recv + (A_norm X)_recv

Sharding: clusters are greedily assigned to 8 cores, so intra-cluster edges
are core-local.  Per core, nodes get local column ids with RECEIVING nodes
(intra-in-degree > 0, ~18% of nodes) first, rank-ordered by unique in-degree
descending.  Everything crossing HBM is bf16 (the 2e-2 rel-err budget allows
it; measured end-to-end error ~3e-3), which halves DMA traffic — the
bottleneck — and runs the PE at 1 cycle/row instead of fp32's 4.

Shipped per core:

  x_ft    [128, T*128]   feature-major X^T bf16, bulk-loaded in chunks with
                         per-partition runs >= 512B (full DMA line rate).
  gtab    [128, GKT*128] edge rounds' source rows bf16, flat partition-major
                         (3KB/partition contiguous -> one line-rate DMA).
  smalls  [128, 129+RT+..] W | b | per-slot degree counts & multiplicities
                         (counts are small integers, exact in bf16).

The first load/store chunk IS the correction region (RT receiver tiles).
Its output columns are produced purely from the Z path:
Z'^T[:, k-tile] = (self + edge rounds) accumulated in PSUM by bf16 PE
transposes of per-partition-scaled slot-major tiles; the device computes
1/deg and rsqrt(deg) itself from shipped integer counts.  out = W^T @ Z'^T
for that chunk; all other chunks are plain W^T @ X^T.  Rows whose cluster
has no intra-cluster edges are overwritten with the original fp32 X rows on
the host at unshard time (pure data movement, no float math).

Device does all float math; host does integer/index preprocessing and data
layout only.
"""

import itertools

import numpy as np

import concourse.bacc as bacc
import concourse.mybir as mybir
import concourse.tile as tile
from concourse.bass_utils import run_bass_kernel_spmd
from concourse.masks import make_identity

N_CORES = 8
P = 128           # partitions
D = 128           # feature dim
N_CLUSTERS = 64
REST_TILES = 8    # node tiles per non-correction load/store chunk
MM_COLS = 512     # moving-operand columns per matmul (one PSUM bank)
WARMUP_MM = 8     # scratch matmuls to ramp the PE clock during DMA-in

F32 = mybir.dt.float32
BF16 = mybir.dt.bfloat16


# --------------------------------------------------------------------------
# Bass program (SPMD across cores; one program, per-core data)
# --------------------------------------------------------------------------

def build_program(T, RT, KTS, has_bias, mask_cols=0):
    """T: node tiles; RT: receiver tiles; KTS: per-round tile counts
    (round 0 = self term, kt=RT; rounds 1.. = edge rounds).  mask_cols is
    accepted for interface compatibility; masked rows are fixed up on the
    host."""
    R = len(KTS)
    NC = T * P
    GKT = sum(KTS[1:])      # gather-table tiles (edge rounds only)
    zc = RT * P             # correction columns == chunk 0
    # smalls layout: W (128 cols) | b (1 col, if bias) | degd (RT) |
    #                per round r>=1: wm_r (kt) | degs_r (kt)
    sf_cols = (1 if has_bias else 0) + RT + 2 * GKT
    s_cols = D + sf_cols
    nc = bacc.Bacc("TRN2", target_bir_lowering=False, debug=False)

    x_ft = nc.declare_dram_parameter("x_ft", [P, NC], BF16, isOutput=False)
    smalls = nc.declare_dram_parameter("smalls", [P, s_cols], BF16, isOutput=False)
    if GKT:
        gtab = nc.declare_dram_parameter("gtab", [P, GKT * D], BF16, isOutput=False)
    out_ft = nc.declare_dram_parameter("out_ft", [P, NC], BF16, isOutput=True)

    # chunking: chunk 0 = correction region (RT tiles), rest REST_TILES each
    ch_tiles = [RT] if RT else []
    rem = T - RT
    while rem > 0:
        t = min(REST_TILES, rem)
        ch_tiles.append(t)
        rem -= t
    n_ch = len(ch_tiles)
    ch_cols = [t * P for t in ch_tiles]
    ch_off = [0]
    for t in ch_tiles[:-1]:
        ch_off.append(ch_off[-1] + t * P)
    c_corr = 0 if RT else -1            # chunk index of the correction region

    g_off = [sum(KTS[1:r]) for r in range(R)]   # tile offset per round

    with tile.TileContext(nc) as tc:
        with (
            tc.tile_pool(name="const", bufs=1) as cpool,
            tc.tile_pool(name="xbuf", bufs=1) as xpool,
            tc.tile_pool(name="stage", bufs=1) as spool,
            tc.tile_pool(name="gbuf", bufs=1) as gpool,
            tc.tile_pool(name="tmp", bufs=4) as mpool,
            tc.tile_pool(name="zt", bufs=1) as zpool,
            tc.tile_pool(name="mmp", bufs=4, space="PSUM") as mpsum,
            tc.tile_pool(name="trp", bufs=2, space="PSUM") as tpsum,
        ):
            # ---- bulk loads: smalls + all x chunks on the sync/HWDGE queue,
            #      gather table on the Pool/SWDGE queue (parallel desc-gen) ----
            sm_sb = cpool.tile([P, s_cols], BF16, tag="smalls")
            nc.sync.dma_start(out=sm_sb[:], in_=smalls[:])
            x_ch = []
            for c in range(n_ch):
                xt = xpool.tile([P, ch_cols[c]], BF16, tag=f"x{c}")
                nc.sync.dma_start(
                    out=xt[:], in_=x_ft[:, ch_off[c]:ch_off[c] + ch_cols[c]]
                )
                x_ch.append(xt)
            if GKT:
                g_all = gpool.tile([P, GKT * P], BF16, tag="gall")
                nc.gpsimd.dma_start(out=g_all[:], in_=gtab[:])

            def g_tile(r, k):
                o = (g_off[r] + k) * P
                return g_all[:, o:o + P]

            ident = cpool.tile([P, P], BF16, tag="ident")
            make_identity(nc, ident[:])

            # ---- PE warmup: cheap matmuls on scratch during the initial
            #      DMA window, so real matmuls run at full clock ----
            wu = cpool.tile([P, P], BF16, tag="wu")
            nc.vector.memset(wu[:], 1.0)
            for _ in range(WARMUP_MM):
                wu_ps = tpsum.tile([P, P], BF16, tag="xtp")
                nc.tensor.matmul(
                    out=wu_ps[:], lhsT=wu[:], rhs=ident[:],
                    is_transpose=True, start=True, stop=True,
                )

            w_sb = sm_sb[:, 0:D]

            # fp32 copy of the small scalar payload (counts, bias);
            # all weight math runs in fp32 from here
            if sf_cols:
                smf = cpool.tile([P, sf_cols], F32, tag="smf")
                nc.vector.tensor_copy(smf[:], sm_sb[:, D:s_cols])
            off = 0
            if has_bias:
                b_sb = smf[:, off:off + 1]
                off += 1
            degd_sb = smf[:, off:off + RT]
            off += RT
            wm_sb, degs_sb = [None], [None]
            for r in range(1, R):
                kt = KTS[r]
                wm_sb.append(smf[:, off:off + kt]); off += kt
                degs_sb.append(smf[:, off:off + kt]); off += kt

            # ---- per-slot weights (fp32 on-chip) ----
            w_rounds = []
            if RT:
                d1 = mpool.tile([P, RT], F32, tag="wprep")
                nc.vector.tensor_scalar_add(d1[:], degd_sb, 1.0)
                dinv = cpool.tile([P, RT], F32, tag="dinv")
                nc.vector.reciprocal(dinv[:], d1[:])
                w_rounds.append(dinv)          # self term: scale by 1/deg
                wd = cpool.tile([P, RT], F32, tag="wd")
                nc.scalar.sqrt(wd[:], dinv[:])
                for r in range(1, R):
                    kt = KTS[r]
                    s1 = mpool.tile([P, kt], F32, tag="wprep")
                    nc.vector.tensor_scalar_add(s1[:], degs_sb[r], 1.0)
                    rec = mpool.tile([P, kt], F32, tag="wprep")
                    nc.vector.reciprocal(rec[:], s1[:])
                    ws = mpool.tile([P, kt], F32, tag="wprep")
                    nc.scalar.sqrt(ws[:], rec[:])
                    wr = cpool.tile([P, kt], F32, tag=f"wr{r}")
                    nc.vector.tensor_mul(wr[:], wm_sb[r], ws[:])
                    nc.vector.tensor_mul(wr[:], wr[:], wd[:, :kt])
                    w_rounds.append(wr)

            staging = [
                spool.tile([P, ch_cols[c]], BF16, name=f"stage{c}", tag=f"s{c}")
                for c in range(n_ch)
            ]

            # engine rotation for elementwise work (scale / copy / evict).
            # Pool/GPSIMD cannot access PSUM on TRN2, so PSUM-touching ops
            # rotate over DVE/ACT only; SBUF-only ops may use Pool too.
            def rot(engines):
                return itertools.cycle(engines)

            def e_scale(eng, out, in_, sc):
                if eng == "v":
                    nc.vector.tensor_scalar_mul(out, in_, sc)
                elif eng == "a":
                    nc.scalar.mul(out, in_, sc)
                else:
                    nc.gpsimd.tensor_scalar_mul(out, in_, sc)

            def e_copy(eng, out, in_):
                if eng == "v":
                    nc.vector.tensor_copy(out, in_)
                elif eng == "a":
                    nc.scalar.copy(out, in_)
                else:
                    nc.gpsimd.tensor_copy(out, in_)

            ev_rot = rot(["v", "a"])      # eviction: PSUM -> SBUF
            ps_rot = rot(["a", "v"])      # Z-path PSUM-side scale/copy
            sb_rot = rot(["p", "v", "a"])  # Z-path SBUF-only scales

            def evict(c, o, w_, ps):
                dst = staging[c][:, o:o + w_]
                eng = next(ev_rot)
                if has_bias:
                    if eng == "v":
                        nc.vector.tensor_scalar_add(dst, ps[:, :w_], b_sb)
                    elif eng == "a":
                        nc.scalar.add(dst, ps[:, :w_], b_sb)
                    else:
                        nc.gpsimd.tensor_scalar_add(dst, ps[:, :w_], b_sb)
                else:
                    e_copy(eng, dst, ps[:, :w_])

            def main_mm(c, rhs_tile):
                """All matmul sub-chunks of chunk c against rhs_tile."""
                o = 0
                while o < ch_cols[c]:
                    w_ = min(MM_COLS, ch_cols[c] - o)
                    ps = mpsum.tile([P, MM_COLS], F32, tag="mm")
                    nc.tensor.matmul(
                        out=ps[:, :w_], lhsT=w_sb, rhs=rhs_tile[:, o:o + w_],
                        start=True, stop=True,
                    )
                    evict(c, o, w_, ps)
                    o += w_

            # ---- Z phase A: self-term transposes + scales (needs only
            #      chunk 0 + weights; runs while later chunks stream in).
            #      S = Z' slot-major, accumulated in SBUF (PSUM transposes
            #      only accumulate in fp32, so SBUF adds instead) ----
            sg0 = None
            if RT:
                sg0 = zpool.tile([P, zc], BF16, tag="sg0")
                for k in range(RT):
                    xp = tpsum.tile([P, P], BF16, tag="xtp")
                    nc.tensor.transpose(
                        out=xp[:], in_=x_ch[0][:, k * P:(k + 1) * P],
                        identity=ident[:],
                    )
                    e_scale(next(ps_rot), sg0[:, k * P:(k + 1) * P], xp[:],
                            dinv[:, k:k + 1])

            # ---- first wave of plain matmuls while the gather table lands ----
            first_wave = [c for c in range(n_ch) if c != c_corr][:2]
            for c in first_wave:
                main_mm(c, x_ch[c])

            # ---- Z phase B: fused scale+accumulate of the edge rounds into
            #      S (SBUF, slot-major), then one bf16 PE transpose per tile
            #      into zt = Z'^T feature-major ----
            zt_sb = None
            if RT:
                zt_sb = zpool.tile([P, zc], BF16, tag="zt")
                for k in range(RT):
                    s_k = sg0[:, k * P:(k + 1) * P]
                    for r in range(1, R):
                        if k >= KTS[r]:
                            continue
                        nc.vector.scalar_tensor_tensor(
                            out=s_k, in0=g_tile(r, k),
                            scalar=w_rounds[r][:, k:k + 1], in1=s_k,
                            op0=mybir.AluOpType.mult,
                            op1=mybir.AluOpType.add,
                        )
                    zp = tpsum.tile([P, P], BF16, tag="ztp")
                    nc.tensor.transpose(
                        out=zp[:], in_=s_k, identity=ident[:],
                    )
                    e_copy(next(ps_rot), zt_sb[:, k * P:(k + 1) * P], zp[:])

            # ---- correction chunk: out cols = W^T @ Z'^T ----
            if RT:
                main_mm(c_corr, zt_sb)

            # ---- remaining plain matmuls ----
            for c in range(n_ch):
                if c == c_corr or c in first_wave:
                    continue
                main_mm(c, x_ch[c])

            # ---- bulk output store, in expected completion order ----
            rest = [c for c in range(n_ch) if c != c_corr]
            order = rest[:3] + ([c_corr] if RT else []) + rest[3:]
            for c in order:
                nc.sync.dma_start(
                    out=out_ft[:, ch_off[c]:ch_off[c] + ch_cols[c]],
                    in_=staging[c][:, :ch_cols[c]],
                )

    nc.finalize()
    return nc


# --------------------------------------------------------------------------
# Host-side sharding / index preprocessing (integer ops + layout only)
# --------------------------------------------------------------------------

def _bf16(a):
    import ml_dtypes
    return np.ascontiguousarray(a.astype(ml_dtypes.bfloat16))


def _prepare(X, W, b, cluster_assignment, edge_index):
    N = X.shape[0]
    has_bias = bool(np.any(b))
    ca = np.asarray(cluster_assignment).astype(np.int64)
    ei = np.asarray(edge_index).astype(np.int64)
    n_cl = max(N_CLUSTERS, int(ca.max()) + 1 if ca.size else 1)
    src, dst = ei[0], ei[1]
    intra = ca[src] == ca[dst]
    isrc, idst = src[intra], dst[intra]

    degcnt = np.bincount(idst, minlength=N).astype(np.int64)
    cluster_edges = np.bincount(ca[isrc], minlength=n_cl)
    cluster_has = cluster_edges > 0
    node_masked = ~cluster_has[ca]          # rows that keep raw X

    # dedup multi-edges -> (usrc, udst, mult)
    if len(isrc):
        pair = isrc * N + idst
        upair, mult = np.unique(pair, return_counts=True)
        usrc, udst = upair // N, upair % N
    else:
        usrc = udst = mult = np.zeros(0, dtype=np.int64)
    udeg = np.bincount(udst, minlength=N).astype(np.int64)

    # greedy cluster -> core assignment (balance node counts)
    csize = np.bincount(ca, minlength=n_cl)
    order = np.argsort(-csize, kind="stable")
    loads = np.zeros(N_CORES, dtype=np.int64)
    cl_core = np.zeros(n_cl, dtype=np.int64)
    for c in order:
        k = int(loads.argmin())
        cl_core[c] = k
        loads[k] += csize[c]
    node_core = cl_core[ca]

    T = int(np.ceil(loads.max() / P))

    # per-core local node order: [recv by udeg desc][nonrecv incl masked]
    cores = []
    max_nrecv = 0
    max_rounds = 0
    for k in range(N_CORES):
        nodes_k = np.where(node_core == k)[0]
        deg_k = udeg[nodes_k]
        recv = nodes_k[deg_k > 0]
        recv = recv[np.argsort(-udeg[recv], kind="stable")]
        nonrecv = nodes_k[deg_k == 0]
        masked_k = nodes_k[node_masked[nodes_k]]
        max_nrecv = max(max_nrecv, len(recv))
        if len(recv):
            max_rounds = max(max_rounds, int(udeg[recv].max()))
        cores.append(dict(recv=recv, nonrecv=nonrecv, masked=masked_k))

    RT = int(np.ceil(max_nrecv / P)) if max_nrecv else 0
    R = (1 + max_rounds) if RT else 0      # round 0 = self term

    # per-round tile counts (unified across cores); round 0 covers all recv
    KTS = [RT] if RT else []
    for r in range(1, R):
        m_r = 0
        for k in range(N_CORES):
            m_r = max(m_r, int((udeg[cores[k]["recv"]] > r - 1).sum()))
        KTS.append(int(np.ceil(m_r / P)))
    GKT = sum(KTS[1:])

    Xf = np.ascontiguousarray(np.asarray(X, dtype=np.float32))
    Wf = np.ascontiguousarray(np.asarray(W, dtype=np.float32))
    bf = np.asarray(b, dtype=np.float32).reshape(-1)
    in_maps = []
    for k in range(N_CORES):
        ck = cores[k]
        recv, nonrecv = ck["recv"], ck["nonrecv"]
        n_recv = len(recv)
        NCk = T * P
        # local (column) ids
        order_all = np.concatenate([recv, nonrecv])
        lid = np.full(N, -1, dtype=np.int64)
        lid[order_all] = np.arange(len(order_all))
        ck["lid"] = lid
        ck["local_nodes"] = order_all

        x_loc = np.zeros((NCk, D), dtype=np.float32)
        x_loc[lid[order_all]] = Xf[order_all]
        m = dict(x_ft=_bf16(np.ascontiguousarray(x_loc.T)))

        # smalls: W | b | degd | per-round wm, degs
        sm = [Wf, bf[:, None]] if has_bias else [Wf]
        gt = np.zeros((P, GKT, D), dtype=np.float32)   # [p, tile, feat]
        if RT:
            dd = np.zeros((P, RT), dtype=np.float32)
            ranks = np.arange(n_recv)
            pp0, tt0 = ranks % P, ranks // P
            dd[pp0, tt0] = degcnt[recv].astype(np.float32)
            sm.append(dd)

        # unique intra edges whose dst lives on this core
        sel = node_core[udst] == k
        es, ed, em = usrc[sel], udst[sel], mult[sel]
        rank_of = np.full(N, -1, dtype=np.int64)
        rank_of[recv] = np.arange(n_recv)
        rnk = rank_of[ed]
        o = np.argsort(rnk, kind="stable")
        es, em, rnk = es[o], em[o], rnk[o]
        if len(rnk):
            starts = np.r_[0, np.flatnonzero(np.diff(rnk)) + 1]
            grp = np.repeat(np.arange(len(starts)), np.diff(np.r_[starts, len(rnk)]))
            seq = np.arange(len(rnk)) - starts[grp]
        else:
            seq = np.zeros(0, dtype=np.int64)

        g_off = [sum(KTS[1:r]) for r in range(R)]
        for r in range(1, R):
            kt = KTS[r]
            wm = np.zeros((P, kt), dtype=np.float32)
            dg = np.zeros((P, kt), dtype=np.float32)
            e_r = seq == (r - 1)
            rr = rnk[e_r]
            pp, tt = rr % P, rr // P
            gt[pp, g_off[r] + tt] = Xf[es[e_r]]
            wm[pp, tt] = em[e_r].astype(np.float32)
            dg[pp, tt] = degcnt[es[e_r]].astype(np.float32)
            sm.append(wm)
            sm.append(dg)

        m["smalls"] = _bf16(np.concatenate(sm, axis=1))
        if GKT:
            m["gtab"] = _bf16(gt.reshape(P, GKT * D))
        in_maps.append(m)

    meta = dict(T=T, RT=RT, KTS=KTS, mask_cols=0,
                cores=cores, N=N, has_bias=has_bias)
    return in_maps, meta


def _finish(results, meta, Xf):
    N = meta["N"]
    out = np.zeros((N, D), dtype=np.float32)
    for k in range(N_CORES):
        ck = meta["cores"][k]
        nodes = ck["local_nodes"]
        rows = ck["lid"][nodes]
        out[nodes] = results[k]["out_ft"].T[rows].astype(np.float32)
        if len(ck["masked"]):
            out[ck["masked"]] = Xf[ck["masked"]]
    return out


def _run(inputs, trace=False, trace_kwargs=None):
    X = np.asarray(inputs["X"], dtype=np.float32)
    W = np.asarray(inputs["W"], dtype=np.float32)
    b = np.asarray(inputs["b"], dtype=np.float32)
    in_maps, meta = _prepare(
        X, W, b, inputs["cluster_assignment"], inputs["edge_index"]
    )
    nc = build_program(meta["T"], meta["RT"], meta["KTS"], meta["has_bias"],
                       meta["mask_cols"])
    res = run_bass_kernel_spmd(
        nc, in_maps, list(range(N_CORES)), trace=trace,
        **(dict(trace_kwargs=trace_kwargs) if trace_kwargs else {}),
    )
    out = _finish(res.results, meta, X)
    return out, res


def kernel(**inputs) -> np.ndarray:
    out, _ = _run(inputs)
    return out


# revision 9
# speedup vs baseline: 1.2999x; 1.0064x over previous
"""Cluster-GCN layer on 8 Trainium2 NeuronCores (Bass/Tile), bf16 edition.

Math (see reference): with A_norm the intra-cluster normalized adjacency and
deg = intra-in-degree + 1,

    out = A_norm @ (X W) + diag(1/deg) (X W) + b        (masked rows keep X)
        = (X' ) @ W + b   where  X'        = X                  (non-recv cols)
                                 X'_recv   = diag(1/deg) X_recv + (A_norm X)_recv

Sharding: clusters are greedily assigned to 8 cores, so intra-cluster edges
are core-local.  Per core, nodes get local column ids with RECEIVING nodes
(intra-in-degree > 0, ~18% of nodes) first, rank-ordered by unique in-degree
descending.  Everything crossing HBM is bf16 (the 2e-2 rel-err budget allows
it; measured end-to-end error ~3e-3), which halves DMA traffic — the
bottleneck — and runs the PE at 1 cycle/row instead of fp32's 4.

Shipped per core:

  x_ft    [128, T*128]   feature-major X^T bf16, bulk-loaded in chunks with
                         per-partition runs >= 512B (full DMA line rate).
  gtab    [128, GKT*128] edge rounds' source rows bf16, flat partition-major
                         (3KB/partition contiguous -> one line-rate DMA).
  smalls  [128, 129+RT+..] W | b | per-slot degree counts & multiplicities
                         (counts are small integers, exact in bf16).

The first load/store chunk IS the correction region (RT receiver tiles).
Its output columns are produced purely from the Z path:
Z'^T[:, k-tile] = (self + edge rounds) accumulated in PSUM by bf16 PE
transposes of per-partition-scaled slot-major tiles; the device computes
1/deg and rsqrt(deg) itself from shipped integer counts.  out = W^T @ Z'^T
for that chunk; all other chunks are plain W^T @ X^T.  Rows whose cluster
has no intra-cluster edges are overwritten with the original fp32 X rows on
the host at unshard time (pure data movement, no float math).

Device does all float math; host does integer/index preprocessing and data
layout only.
"""

import itertools

import numpy as np

import concourse.bacc as bacc
import concourse.mybir as mybir
import concourse.tile as tile
from concourse.bass_utils import run_bass_kernel_spmd
from concourse.masks import make_identity

N_CORES = 8
P = 128           # partitions
D = 128           # feature dim
N_CLUSTERS = 64
REST_TILES = 8    # node tiles per non-correction load/store chunk
MM_COLS = 512     # moving-operand columns per matmul (one PSUM bank)
WARMUP_MM = 8     # scratch matmuls to ramp the PE clock during DMA-in

F32 = mybir.dt.float32
BF16 = mybir.dt.bfloat16


# --------------------------------------------------------------------------
# Bass program (SPMD across cores; one program, per-core data)
# --------------------------------------------------------------------------

def build_program(T, RT, KTS, has_bias, mask_cols=0):
    """T: node tiles; RT: receiver tiles; KTS: per-round tile counts
    (round 0 = self term, kt=RT; rounds 1.. = edge rounds).  mask_cols is
    accepted for interface compatibility; masked rows are fixed up on the
    host."""
    R = len(KTS)
    NC = T * P
    GKT = sum(KTS[1:])      # gather-table tiles (edge rounds only)
    zc = RT * P             # correction columns == chunk 0
    # smalls layout: W (128 cols) | b (1 col, if bias) | degd (RT) |
    #                per round r>=1: wm_r (kt) | degs_r (kt)
    sf_cols = (1 if has_bias else 0) + RT + 2 * GKT
    s_cols = D + sf_cols
    nc = bacc.Bacc("TRN2", target_bir_lowering=False, debug=False)

    x_ft = nc.declare_dram_parameter("x_ft", [P, NC], BF16, isOutput=False)
    smalls = nc.declare_dram_parameter("smalls", [P, s_cols], BF16, isOutput=False)
    if GKT:
        gtab = nc.declare_dram_parameter("gtab", [P, GKT * D], BF16, isOutput=False)
    out_ft = nc.declare_dram_parameter("out_ft", [P, NC], BF16, isOutput=True)

    # chunking: chunk 0 = correction region (RT tiles), rest REST_TILES each
    ch_tiles = [RT] if RT else []
    rem = T - RT
    while rem > 0:
        t = min(REST_TILES, rem)
        ch_tiles.append(t)
        rem -= t
    n_ch = len(ch_tiles)
    ch_cols = [t * P for t in ch_tiles]
    ch_off = [0]
    for t in ch_tiles[:-1]:
        ch_off.append(ch_off[-1] + t * P)
    c_corr = 0 if RT else -1            # chunk index of the correction region

    g_off = [sum(KTS[1:r]) for r in range(R)]   # tile offset per round

    with tile.TileContext(nc) as tc:
        with (
            tc.tile_pool(name="const", bufs=1) as cpool,
            tc.tile_pool(name="xbuf", bufs=1) as xpool,
            tc.tile_pool(name="stage", bufs=1) as spool,
            tc.tile_pool(name="gbuf", bufs=1) as gpool,
            tc.tile_pool(name="tmp", bufs=4) as mpool,
            tc.tile_pool(name="zt", bufs=1) as zpool,
            tc.tile_pool(name="mmp", bufs=4, space="PSUM") as mpsum,
            tc.tile_pool(name="trp", bufs=2, space="PSUM") as tpsum,
        ):
            # ---- bulk loads: smalls + all x chunks on the sync/HWDGE queue,
            #      gather table on the Pool/SWDGE queue (parallel desc-gen) ----
            x_ch = [None] * n_ch
            sm_sb = cpool.tile([P, s_cols], BF16, tag="smalls")
            load_order = ([c_corr] if RT else []) + ["sm"] + \
                [c for c in range(n_ch) if c != c_corr] + ([] if RT else ["sm_skip"])
            for c in load_order:
                if c == "sm":
                    nc.sync.dma_start(out=sm_sb[:], in_=smalls[:])
                    continue
                if c == "sm_skip":
                    continue
                xt = xpool.tile([P, ch_cols[c]], BF16, name=f"x{c}", tag=f"x{c}")
                nc.sync.dma_start(
                    out=xt[:], in_=x_ft[:, ch_off[c]:ch_off[c] + ch_cols[c]]
                )
                x_ch[c] = xt
            if not RT:
                nc.sync.dma_start(out=sm_sb[:], in_=smalls[:])
            if GKT:
                g_all = gpool.tile([P, GKT * P], BF16, tag="gall")
                nc.gpsimd.dma_start(out=g_all[:], in_=gtab[:])

            def g_tile(r, k):
                o = (g_off[r] + k) * P
                return g_all[:, o:o + P]

            ident = cpool.tile([P, P], BF16, tag="ident")
            make_identity(nc, ident[:])

            # ---- PE warmup: cheap matmuls on scratch during the initial
            #      DMA window, so real matmuls run at full clock ----
            wu = cpool.tile([P, P], BF16, tag="wu")
            nc.vector.memset(wu[:], 1.0)
            for _ in range(WARMUP_MM):
                wu_ps = tpsum.tile([P, P], BF16, tag="xtp")
                nc.tensor.matmul(
                    out=wu_ps[:], lhsT=wu[:], rhs=ident[:],
                    is_transpose=True, start=True, stop=True,
                )

            w_sb = sm_sb[:, 0:D]

            # fp32 copy of the small scalar payload (counts, bias);
            # all weight math runs in fp32 from here
            if sf_cols:
                smf = cpool.tile([P, sf_cols], F32, tag="smf")
                nc.vector.tensor_copy(smf[:], sm_sb[:, D:s_cols])
            off = 0
            if has_bias:
                b_sb = smf[:, off:off + 1]
                off += 1
            degd_sb = smf[:, off:off + RT]
            off += RT
            wm_sb, degs_sb = [None], [None]
            for r in range(1, R):
                kt = KTS[r]
                wm_sb.append(smf[:, off:off + kt]); off += kt
                degs_sb.append(smf[:, off:off + kt]); off += kt

            # ---- per-slot weights (fp32 on-chip) ----
            w_rounds = []
            if RT:
                d1 = mpool.tile([P, RT], F32, tag="wprep")
                nc.vector.tensor_scalar_add(d1[:], degd_sb, 1.0)
                dinv = cpool.tile([P, RT], F32, tag="dinv")
                nc.vector.reciprocal(dinv[:], d1[:])
                w_rounds.append(dinv)          # self term: scale by 1/deg
                wd = cpool.tile([P, RT], F32, tag="wd")
                nc.scalar.sqrt(wd[:], dinv[:])
                for r in range(1, R):
                    kt = KTS[r]
                    s1 = mpool.tile([P, kt], F32, tag="wprep")
                    nc.vector.tensor_scalar_add(s1[:], degs_sb[r], 1.0)
                    rec = mpool.tile([P, kt], F32, tag="wprep")
                    nc.vector.reciprocal(rec[:], s1[:])
                    ws = mpool.tile([P, kt], F32, tag="wprep")
                    nc.scalar.sqrt(ws[:], rec[:])
                    wr = cpool.tile([P, kt], F32, tag=f"wr{r}")
                    nc.vector.tensor_mul(wr[:], wm_sb[r], ws[:])
                    nc.vector.tensor_mul(wr[:], wr[:], wd[:, :kt])
                    w_rounds.append(wr)

            staging = [
                spool.tile([P, ch_cols[c]], BF16, name=f"stage{c}", tag=f"s{c}")
                for c in range(n_ch)
            ]

            # engine rotation for elementwise work (scale / copy / evict).
            # Pool/GPSIMD cannot access PSUM on TRN2, so PSUM-touching ops
            # rotate over DVE/ACT only; SBUF-only ops may use Pool too.
            def rot(engines):
                return itertools.cycle(engines)

            def e_scale(eng, out, in_, sc):
                if eng == "v":
                    nc.vector.tensor_scalar_mul(out, in_, sc)
                elif eng == "a":
                    nc.scalar.mul(out, in_, sc)
                else:
                    nc.gpsimd.tensor_scalar_mul(out, in_, sc)

            def e_copy(eng, out, in_):
                if eng == "v":
                    nc.vector.tensor_copy(out, in_)
                elif eng == "a":
                    nc.scalar.copy(out, in_)
                else:
                    nc.gpsimd.tensor_copy(out, in_)

            ev_rot = rot(["v", "a"])      # eviction: PSUM -> SBUF
            ps_rot = rot(["a", "v"])      # Z-path PSUM-side scale/copy
            sb_rot = rot(["p", "v", "a"])  # Z-path SBUF-only scales

            def evict(c, o, w_, ps):
                dst = staging[c][:, o:o + w_]
                eng = next(ev_rot)
                if has_bias:
                    if eng == "v":
                        nc.vector.tensor_scalar_add(dst, ps[:, :w_], b_sb)
                    elif eng == "a":
                        nc.scalar.add(dst, ps[:, :w_], b_sb)
                    else:
                        nc.gpsimd.tensor_scalar_add(dst, ps[:, :w_], b_sb)
                else:
                    e_copy(eng, dst, ps[:, :w_])

            def main_mm(c, rhs_tile):
                """All matmul sub-chunks of chunk c against rhs_tile."""
                o = 0
                while o < ch_cols[c]:
                    w_ = min(MM_COLS, ch_cols[c] - o)
                    ps = mpsum.tile([P, MM_COLS], F32, tag="mm")
                    nc.tensor.matmul(
                        out=ps[:, :w_], lhsT=w_sb, rhs=rhs_tile[:, o:o + w_],
                        start=True, stop=True,
                    )
                    evict(c, o, w_, ps)
                    o += w_

            # ---- Z phase A: self-term transposes + scales (needs only
            #      chunk 0 + weights; runs while later chunks stream in).
            #      S = Z' slot-major, accumulated in SBUF (PSUM transposes
            #      only accumulate in fp32, so SBUF adds instead) ----
            sg0 = None
            if RT:
                sg0 = zpool.tile([P, zc], BF16, tag="sg0")
                for k in range(RT):
                    xp = tpsum.tile([P, P], BF16, tag="xtp")
                    nc.tensor.transpose(
                        out=xp[:], in_=x_ch[0][:, k * P:(k + 1) * P],
                        identity=ident[:],
                    )
                    e_scale(next(ps_rot), sg0[:, k * P:(k + 1) * P], xp[:],
                            dinv[:, k:k + 1])

            # ---- Z accumulate: fused scale+add of the edge rounds into
            #      S = sg0 (SBUF, slot-major) on DVE; emitted before the
            #      plain-mm evictions so the DVE queue has no head-of-line
            #      stall (these only need gtab + weights) ----
            if RT:
                for k in range(RT):
                    s_k = sg0[:, k * P:(k + 1) * P]
                    for r in range(1, R):
                        if k >= KTS[r]:
                            continue
                        nc.vector.scalar_tensor_tensor(
                            out=s_k, in0=g_tile(r, k),
                            scalar=w_rounds[r][:, k:k + 1], in1=s_k,
                            op0=mybir.AluOpType.mult,
                            op1=mybir.AluOpType.add,
                        )

            # ---- all plain matmuls (data-ready order) ----
            for c in range(n_ch):
                if c == c_corr:
                    continue
                main_mm(c, x_ch[c])

            # ---- Z phase B: one bf16 PE transpose per receiver tile into
            #      zt = Z'^T feature-major, then the correction matmuls ----
            zt_sb = None
            if RT:
                zt_sb = zpool.tile([P, zc], BF16, tag="zt")
                for k in range(RT):
                    zp = tpsum.tile([P, P], BF16, tag="ztp")
                    nc.tensor.transpose(
                        out=zp[:], in_=sg0[:, k * P:(k + 1) * P],
                        identity=ident[:],
                    )
                    e_copy(next(ps_rot), zt_sb[:, k * P:(k + 1) * P], zp[:])
                main_mm(c_corr, zt_sb)

            # ---- bulk output store: plain chunks as they finish, the
            #      correction chunk last ----
            order = [c for c in range(n_ch) if c != c_corr] + \
                ([c_corr] if RT else [])
            for c in order:
                nc.sync.dma_start(
                    out=out_ft[:, ch_off[c]:ch_off[c] + ch_cols[c]],
                    in_=staging[c][:, :ch_cols[c]],
                )

    nc.finalize()
    return nc


# --------------------------------------------------------------------------
# Host-side sharding / index preprocessing (integer ops + layout only)
# --------------------------------------------------------------------------

def _bf16(a):
    import ml_dtypes
    return np.ascontiguousarray(a.astype(ml_dtypes.bfloat16))


def _prepare(X, W, b, cluster_assignment, edge_index):
    N = X.shape[0]
    has_bias = bool(np.any(b))
    ca = np.asarray(cluster_assignment).astype(np.int64)
    ei = np.asarray(edge_index).astype(np.int64)
    n_cl = max(N_CLUSTERS, int(ca.max()) + 1 if ca.size else 1)
    src, dst = ei[0], ei[1]
    intra = ca[src] == ca[dst]
    isrc, idst = src[intra], dst[intra]

    degcnt = np.bincount(idst, minlength=N).astype(np.int64)
    cluster_edges = np.bincount(ca[isrc], minlength=n_cl)
    cluster_has = cluster_edges > 0
    node_masked = ~cluster_has[ca]          # rows that keep raw X

    # dedup multi-edges -> (usrc, udst, mult)
    if len(isrc):
        pair = isrc * N + idst
        upair, mult = np.unique(pair, return_counts=True)
        usrc, udst = upair // N, upair % N
    else:
        usrc = udst = mult = np.zeros(0, dtype=np.int64)
    udeg = np.bincount(udst, minlength=N).astype(np.int64)

    # greedy cluster -> core assignment (balance node counts)
    csize = np.bincount(ca, minlength=n_cl)
    order = np.argsort(-csize, kind="stable")
    loads = np.zeros(N_CORES, dtype=np.int64)
    cl_core = np.zeros(n_cl, dtype=np.int64)
    for c in order:
        k = int(loads.argmin())
        cl_core[c] = k
        loads[k] += csize[c]
    node_core = cl_core[ca]

    T = int(np.ceil(loads.max() / P))

    # per-core local node order: [recv by udeg desc][nonrecv incl masked]
    cores = []
    max_nrecv = 0
    max_rounds = 0
    for k in range(N_CORES):
        nodes_k = np.where(node_core == k)[0]
        deg_k = udeg[nodes_k]
        recv = nodes_k[deg_k > 0]
        recv = recv[np.argsort(-udeg[recv], kind="stable")]
        nonrecv = nodes_k[deg_k == 0]
        masked_k = nodes_k[node_masked[nodes_k]]
        max_nrecv = max(max_nrecv, len(recv))
        if len(recv):
            max_rounds = max(max_rounds, int(udeg[recv].max()))
        cores.append(dict(recv=recv, nonrecv=nonrecv, masked=masked_k))

    RT = int(np.ceil(max_nrecv / P)) if max_nrecv else 0
    R = (1 + max_rounds) if RT else 0      # round 0 = self term

    # per-round tile counts (unified across cores); round 0 covers all recv
    KTS = [RT] if RT else []
    for r in range(1, R):
        m_r = 0
        for k in range(N_CORES):
            m_r = max(m_r, int((udeg[cores[k]["recv"]] > r - 1).sum()))
        KTS.append(int(np.ceil(m_r / P)))
    GKT = sum(KTS[1:])

    Xf = np.ascontiguousarray(np.asarray(X, dtype=np.float32))
    Wf = np.ascontiguousarray(np.asarray(W, dtype=np.float32))
    bf = np.asarray(b, dtype=np.float32).reshape(-1)
    in_maps = []
    for k in range(N_CORES):
        ck = cores[k]
        recv, nonrecv = ck["recv"], ck["nonrecv"]
        n_recv = len(recv)
        NCk = T * P
        # local (column) ids
        order_all = np.concatenate([recv, nonrecv])
        lid = np.full(N, -1, dtype=np.int64)
        lid[order_all] = np.arange(len(order_all))
        ck["lid"] = lid
        ck["local_nodes"] = order_all

        x_loc = np.zeros((NCk, D), dtype=np.float32)
        x_loc[lid[order_all]] = Xf[order_all]
        m = dict(x_ft=_bf16(np.ascontiguousarray(x_loc.T)))

        # smalls: W | b | degd | per-round wm, degs
        sm = [Wf, bf[:, None]] if has_bias else [Wf]
        gt = np.zeros((P, GKT, D), dtype=np.float32)   # [p, tile, feat]
        if RT:
            dd = np.zeros((P, RT), dtype=np.float32)
            ranks = np.arange(n_recv)
            pp0, tt0 = ranks % P, ranks // P
            dd[pp0, tt0] = degcnt[recv].astype(np.float32)
            sm.append(dd)

        # unique intra edges whose dst lives on this core
        sel = node_core[udst] == k
        es, ed, em = usrc[sel], udst[sel], mult[sel]
        rank_of = np.full(N, -1, dtype=np.int64)
        rank_of[recv] = np.arange(n_recv)
        rnk = rank_of[ed]
        o = np.argsort(rnk, kind="stable")
        es, em, rnk = es[o], em[o], rnk[o]
        if len(rnk):
            starts = np.r_[0, np.flatnonzero(np.diff(rnk)) + 1]
            grp = np.repeat(np.arange(len(starts)), np.diff(np.r_[starts, len(rnk)]))
            seq = np.arange(len(rnk)) - starts[grp]
        else:
            seq = np.zeros(0, dtype=np.int64)

        g_off = [sum(KTS[1:r]) for r in range(R)]
        for r in range(1, R):
            kt = KTS[r]
            wm = np.zeros((P, kt), dtype=np.float32)
            dg = np.zeros((P, kt), dtype=np.float32)
            e_r = seq == (r - 1)
            rr = rnk[e_r]
            pp, tt = rr % P, rr // P
            gt[pp, g_off[r] + tt] = Xf[es[e_r]]
            wm[pp, tt] = em[e_r].astype(np.float32)
            dg[pp, tt] = degcnt[es[e_r]].astype(np.float32)
            sm.append(wm)
            sm.append(dg)

        m["smalls"] = _bf16(np.concatenate(sm, axis=1))
        if GKT:
            m["gtab"] = _bf16(gt.reshape(P, GKT * D))
        in_maps.append(m)

    meta = dict(T=T, RT=RT, KTS=KTS, mask_cols=0,
                cores=cores, N=N, has_bias=has_bias)
    return in_maps, meta


def _finish(results, meta, Xf):
    N = meta["N"]
    out = np.zeros((N, D), dtype=np.float32)
    for k in range(N_CORES):
        ck = meta["cores"][k]
        nodes = ck["local_nodes"]
        rows = ck["lid"][nodes]
        out[nodes] = results[k]["out_ft"].T[rows].astype(np.float32)
        if len(ck["masked"]):
            out[ck["masked"]] = Xf[ck["masked"]]
    return out


def _run(inputs, trace=False, trace_kwargs=None):
    X = np.asarray(inputs["X"], dtype=np.float32)
    W = np.asarray(inputs["W"], dtype=np.float32)
    b = np.asarray(inputs["b"], dtype=np.float32)
    in_maps, meta = _prepare(
        X, W, b, inputs["cluster_assignment"], inputs["edge_index"]
    )
    nc = build_program(meta["T"], meta["RT"], meta["KTS"], meta["has_bias"],
                       meta["mask_cols"])
    res = run_bass_kernel_spmd(
        nc, in_maps, list(range(N_CORES)), trace=trace,
        **(dict(trace_kwargs=trace_kwargs) if trace_kwargs else {}),
    )
    out = _finish(res.results, meta, X)
    return out, res


def kernel(**inputs) -> np.ndarray:
    out, _ = _run(inputs)
    return out
